# revision 20
# baseline (speedup 1.0000x reference)
"""CCMLite kernel for Trainium2: GroupNorm(affine=False) + low-rank channel mix.

out = x_norm + u @ (v^T @ x_norm) + shift, with x_norm = groupnorm(x).

Sharding: data-parallel over batch B=16 across 8 cores (2 batch elems/core).

DMA floor is ~24us/core (8.4 MB at ~360 GB/s). DVE/ACT are the only engines
that can read PSUM; latency ladders (stats -> group chain -> vs) gate each
batch, so they are kept short and high-priority:
  - all x loads issue up-front on the sync ring; the last tile of each batch
    is split in half so the gating stats start earlier
  - stats ops are sliced to 1024 columns so chain rungs never wait long
    behind bulk ops; s1 via DVE tensor_reduce (runs in DVE's idle load
    window) or ACT Copy+accum; s2 via ACT Square+accum or DVE STT+accum
  - group chain: 1/GPC is folded into the host-side mask so the gs matmul
    yields group means/E[x^2] directly; Rsqrt fuses sqrt+reciprocal
  - stage A (vtx = vs^T x) packs 4 rank-12 strips per PSUM tile via
    tile_position (they pipeline in distinct PE column groups); lhsT is
    zero-padded to 32 cols so strips cover all 128 partitions; the evac op
    subtracts kvec (mean correction) as a per-partition scalar and maps the
    zero rows 32q+12 to +1 (kvecP=-1 there), creating the ones-row that
    carries cst through stage-B matmuls
  - stage B units of [128,1024]: path D = DVE STT (s*x + pm) from PSUM,
    path A = PE diag(s) matmul + ACT Identity+bias, path E = DVE 4x t=s*x,
    ACT evac, Pool TT-add; per-unit stores stream on the sync ring
"""

from contextlib import ExitStack

import numpy as np

import concourse.bass as bass
import concourse.tile as tile
from concourse import bacc, mybir
from concourse.bass_utils import run_bass_kernel_spmd

N_CORES = 8
B, C, H, W = 16, 256, 64, 64
HW = H * W            # 4096
R = 12                # low rank
G = 32                # groups
GPC = C // G          # 8 channels per group
P = 128               # partitions
CB = C // P           # 2 channel blocks
BPC = B // N_CORES    # 2 batch elements per core
EPS = 1e-6
F32 = mybir.dt.float32
F16 = mybir.dt.float16

_MULT = mybir.AluOpType.mult
_ADD = mybir.AluOpType.add
AF = mybir.ActivationFunctionType

# ---- schedule knobs ----
# s2 per (b, cb): 'act' = ACT Square+accum, 'stt' = DVE STT+accum
# (s1 always rides tile_position-packed PE group-sum matmuls)
DEF_S2 = {(0, 0): "act", (0, 1): "act", (1, 0): "act", (1, 1): "stt"}
# stage-B path per batch: 8 chars, unit order u = 2k+cb (k-major)
DEF_PATHS = ("DEADEDAE", "DAEDEDDE")


def build_nc(paths=DEF_PATHS, s2cfg=DEF_S2):
    nc = bacc.Bacc(None, target_bir_lowering=False)
    x_d = nc.dram_tensor("x", [BPC, C, HW], F16, kind="ExternalInput")
    aug_d = nc.dram_tensor("aug", [BPC, P, C], F16, kind="ExternalInput")
    vsh_d = nc.dram_tensor("vsh", [BPC, CB, P, 33], F32, kind="ExternalInput")
    gmaskG_d = nc.dram_tensor("gmaskG", [P, 16], F32, kind="ExternalInput")
    gmask16_d = nc.dram_tensor("gmask16", [P, 32], F16, kind="ExternalInput")
    foldm_d = nc.dram_tensor("foldm", [P, 16], F32, kind="ExternalInput")
    gmaskT_d = nc.dram_tensor("gmaskT", [16, P], F32, kind="ExternalInput")
    ident_d = nc.dram_tensor("ident16", [P, P], F16, kind="ExternalInput")
    kinit_d = nc.dram_tensor("kinit", [16, 1], F32, kind="ExternalInput")
    ones_d = nc.dram_tensor("ones16", [1, 4], F16, kind="ExternalInput")
    out_d = nc.dram_tensor("out", [BPC, C, HW], F16, kind="ExternalOutput")

    with tile.TileContext(nc) as tc, ExitStack() as ctx:
        consts = ctx.enter_context(tc.tile_pool(name="consts", bufs=1))
        xbp = ctx.enter_context(tc.tile_pool(name="xbp", bufs=6))
        junkp = ctx.enter_context(tc.tile_pool(name="junkp", bufs=2))
        outp = ctx.enter_context(tc.tile_pool(name="outp", bufs=6))
        vtp = ctx.enter_context(tc.tile_pool(name="vtp", bufs=4))
        smalls = ctx.enter_context(tc.tile_pool(name="smalls", bufs=2))
        ps_small = ctx.enter_context(
            tc.tile_pool(name="ps_small", bufs=2, space="PSUM"))
        ps_vtx = ctx.enter_context(
            tc.tile_pool(name="ps_vtx", bufs=1, space="PSUM"))
        ps_pm = ctx.enter_context(tc.tile_pool(name="ps_pm", bufs=2, space="PSUM"))

        # ---- consts ----
        gmaskG = consts.tile([P, 16], F32)   # group mask * 1/GPC
        nc.gpsimd.dma_start(out=gmaskG, in_=gmaskG_d[:, :])
        gmask16 = consts.tile([P, 32], F16)  # group mask, zero-padded
        nc.gpsimd.dma_start(out=gmask16, in_=gmask16_d[:, :])
        foldm = consts.tile([P, 16], F32)    # strip fold mask / (GPC*HW)
        nc.gpsimd.dma_start(out=foldm, in_=foldm_d[:, :])
        gmaskT = consts.tile([16, P], F32)
        nc.gpsimd.dma_start(out=gmaskT, in_=gmaskT_d[:, :])
        ident_h = consts.tile([P, P], F16)
        nc.gpsimd.dma_start(out=ident_h, in_=ident_d[:, :])
        kinit = consts.tile([16, 1], F32)
        nc.gpsimd.dma_start(out=kinit, in_=kinit_d[:, :])
        ones14 = consts.tile([1, 4], F16)
        nc.gpsimd.dma_start(out=ones14, in_=ones_d[:, :])
        eps_t = consts.tile([16, 1], F32)
        nc.vector.memset(eps_t, EPS)

        # warm ACT tables first so Sqrt/Square never table-load mid-kernel
        twarm = smalls.tile([16, 1], F32, tag="twarm", bufs=1)
        nc.scalar.activation(out=twarm, in_=eps_t, func=AF.Square)
        nc.scalar.activation(out=twarm, in_=eps_t, func=AF.Sqrt,
                             bias=eps_t[:, 0:1], scale=1.0)
        nc.scalar.activation(out=twarm, in_=eps_t, func=AF.Identity)
        nc.scalar.activation(out=twarm, in_=eps_t, func=AF.Copy)

        # per-batch params on the gpsimd SWDGE ring (Pool is idle early;
        # keeps the scalar sequencer free for the squares)
        augs, vshs = [], []
        for b in range(BPC):
            aug = smalls.tile([P, 2 * P], F16, tag=f"aug{b}", bufs=1)
            nc.gpsimd.dma_start(out=aug, in_=aug_d[b])
            vsh = smalls.tile([P, 66], F32, tag=f"vsh{b}", bufs=1)
            for cb in range(CB):
                nc.gpsimd.dma_start(
                    out=vsh[:, 33 * cb:33 * (cb + 1)], in_=vsh_d[b, cb])
            augs.append(aug)
            vshs.append(vsh)

        # ---- all x loads up-front, sync ring; last tile of each batch split
        # so the tail-gating stats start earlier ----
        # xparts[(b,cb,h)] = list of (tile, col0, width) covering cols 0..2048
        xparts = {}
        for b in range(BPC):
            for cb in range(CB):
                for h in range(2):
                    eng = nc.sync if (cb + h) % 2 == 0 else nc.scalar
                    if cb == 1 and h == 1:
                        parts = []
                        for i in range(2):
                            tb = xbp.tile([P, 1024], F16, tag="xbt2",
                                          bufs=4, name=f"xl{b}{i}")
                            eng.dma_start(
                                out=tb,
                                in_=x_d[b, cb * P:(cb + 1) * P,
                                        h * 2048 + 1024 * i:
                                        h * 2048 + 1024 * (i + 1)])
                            parts.append((tb, 1024 * i, 1024))
                        xparts[(b, cb, h)] = parts
                    else:
                        tb = xbp.tile([P, 2048], F16, tag="xbt",
                                      name=f"x{b}{cb}{h}")
                        eng.dma_start(
                            out=tb,
                            in_=x_d[b, cb * P:(cb + 1) * P,
                                    h * 2048:(h + 1) * 2048])
                        xparts[(b, cb, h)] = [(tb, 0, 2048)]

        def x_ap(b, cb, col0, width):
            h, c = col0 // 2048, col0 % 2048
            for tile_, t0, tw in xparts[(b, cb, h)]:
                if t0 <= c and c + width <= t0 + tw:
                    return tile_[:, c - t0:c - t0 + width]
            raise AssertionError((b, cb, col0, width))

        def x_slices(b, cb, h):
            # 1024-wide (tile, slice) pieces of column range [2048h, 2048h+2048)
            out = []
            for tile_, t0, tw in xparts[(b, cb, h)]:
                for i in range(tw // 1024):
                    out.append(tile_[:, 1024 * i:1024 * (i + 1)])
            return out

        sms = {}     # (b,cb) -> [128,2] f32: col0 rstd, col1 group-mean
        vss = {}     # (b,cb) -> [128,32] f16 (v*s, zero-padded)
        diags = {}   # (b,cb) -> [128,128] f16 diag(s)
        kvsb = {}    # (b,cb) -> [R,1] f32 partial kvec
        kvecPs = {}  # b -> [128,1] f32 strip-replicated kvec (rows 32q+12=-1)
        accs = {}    # (b,cb) -> [128,4] f32 s2 accumulator columns
        naccs = {}   # (b,cb) -> number of s2 cols used
        gxs = {}     # (b,cb) -> [128,512] PSUM packed group partial sums
        csts = {}
        vts = {}
        vtx_ps = {}

        def get_acc(b, cb):
            if (b, cb) not in accs:
                accs[(b, cb)] = smalls.tile(
                    [P, 4], F32, tag=f"acc{b}{cb}", bufs=1, name=f"acc{b}{cb}")
                naccs[(b, cb)] = 0
            return accs[(b, cb)]

        def emit_stats(b, cb, h):
            # s1: packed group-sum matmuls -- strip q of the shared [128,512]
            # PSUM tile accumulates chunks q and q+4 at PE column 32q
            if h == 0:
                gxs[(b, cb)] = ps_small.tile([P, 512], F32, tag="ps",
                                             name=f"gx{b}{cb}")
            gx = gxs[(b, cb)]
            for q in range(4):
                nc.tensor.matmul(
                    gx[32 * q:32 * q + 32, :], lhsT=gmask16,
                    rhs=x_ap(b, cb, 2048 * h + 512 * q, 512),
                    start=(h == 0), stop=(h == 1),
                    tile_position=(0, 32 * q),
                    skip_group_check=True)
            # s2 per natural tile piece
            acc = get_acc(b, cb)
            if s2cfg[(b, cb)] == "act":
                for tile_, t0, tw in xparts[(b, cb, h)]:
                    col = naccs[(b, cb)]
                    naccs[(b, cb)] += 1
                    ja = junkp.tile([P, 2048], F16, tag="ja")
                    nc.scalar.activation(
                        out=ja[:, 0:tw], in_=tile_, func=AF.Square,
                        accum_out=acc[:, col:col + 1])
            else:
                # sliced to 1024 so chain-ladder rungs never wait long
                for sl in x_slices(b, cb, h):
                    col = naccs[(b, cb)]
                    naccs[(b, cb)] += 1
                    jd = junkp.tile([P, 1024], F16, tag="jd")
                    nc.vector.scalar_tensor_tensor(
                        out=jd, in0=sl, scalar=1.0, in1=sl,
                        op0=_MULT, op1=_MULT, accum_out=acc[:, col:col + 1])

        def fold(acc, n, out):
            # out = sum(acc[:, 0:n]) / HW
            if n == 2:
                nc.vector.tensor_scalar(
                    out=out, in0=acc[:, 0:1], scalar1=acc[:, 1:2],
                    scalar2=1.0 / HW, op0=_ADD, op1=_MULT)
            elif n == 3:
                nc.vector.tensor_scalar(
                    out=acc[:, 0:1], in0=acc[:, 0:1], scalar1=acc[:, 1:2],
                    scalar2=acc[:, 2:3], op0=_ADD, op1=_ADD)
                nc.vector.tensor_scalar_mul(
                    out=out, in0=acc[:, 0:1], scalar1=1.0 / HW)
            else:
                assert n == 4
                nc.vector.tensor_scalar(
                    out=acc[:, 0:1], in0=acc[:, 0:1], scalar1=acc[:, 1:2],
                    scalar2=acc[:, 2:3], op0=_ADD, op1=_ADD)
                nc.vector.tensor_scalar(
                    out=out, in0=acc[:, 0:1], scalar1=acc[:, 3:4],
                    scalar2=1.0 / HW, op0=_ADD, op1=_MULT)

        gxrs = {}

        def emit_gxred(b, cb):
            # consume the gx PSUM tile early so the ps ring can rotate
            gxr = smalls.tile([P, 1], F32, tag="gxr", bufs=4,
                              name=f"gxr{b}{cb}")
            nc.vector.tensor_reduce(
                out=gxr, in_=gxs[(b, cb)], axis=mybir.AxisListType.X, op=_ADD)
            gxrs[(b, cb)] = gxr

        def emit_chain(b, cb):
            # ladder: mg(mm) / e2 fold -> gs(mm) -> var -> sqrt ->
            # recip -> bc(mm) -> sm -> vs
            if (b, cb) not in gxrs:
                emit_gxred(b, cb)
            mg = ps_small.tile([16, 1], F32, tag="ps")
            nc.tensor.matmul(mg, lhsT=foldm, rhs=gxrs[(b, cb)],
                             start=True, stop=True)
            acc = accs[(b, cb)]
            msum = smalls.tile([P, 1], F32, tag="msum")
            fold(acc, naccs[(b, cb)], msum)
            gs = ps_small.tile([16, 1], F32, tag="ps")
            nc.tensor.matmul(gs, lhsT=gmaskG, rhs=msum, start=True, stop=True)
            gvals = smalls.tile([16, 2], F32, tag="gvals")
            tmpg = smalls.tile([16, 2], F32, tag="tmpg")
            nc.vector.tensor_copy(out=gvals[:, 1:2], in_=mg)
            nc.vector.tensor_mul(out=tmpg[:, 0:1], in0=gvals[:, 1:2],
                                 in1=gvals[:, 1:2])
            nc.vector.tensor_sub(out=tmpg[:, 1:2], in0=gs,
                                 in1=tmpg[:, 0:1])
            gsd = smalls.tile([16, 1], F32, tag="gsd")
            nc.scalar.activation(
                out=gsd, in_=tmpg[:, 1:2], func=AF.Sqrt,
                bias=eps_t[:, 0:1], scale=1.0)
            nc.vector.reciprocal(out=gvals[:, 0:1], in_=gsd)
            bc = ps_small.tile([P, 2], F32, tag="ps")
            nc.tensor.matmul(bc, lhsT=gmaskT, rhs=gvals, start=True, stop=True)
            sm = smalls.tile([P, 2], F32, tag=f"sm{b}{cb}", bufs=1)
            nc.vector.tensor_copy(out=sm, in_=bc)
            sms[(b, cb)] = sm
            # vs = v * s (fp16, zero-padded cols 12..31 so stage A strips
            # cover all 128 partitions)
            vs = smalls.tile([P, 32], F16, tag=f"vs{b}{cb}", bufs=1)
            nc.vector.tensor_scalar_mul(
                out=vs, in0=vshs[b][:, 33 * cb:33 * cb + 32],
                scalar1=sm[:, 0:1])
            vss[(b, cb)] = vs
            if "A" in paths[b]:
                diag = smalls.tile([P, P], F16, tag=f"diag{b}{cb}", bufs=1)
                nc.vector.tensor_scalar_mul(out=diag, in0=ident_h,
                                            scalar1=sm[:, 0:1])
                diags[(b, cb)] = diag

        def emit_cst(b):
            aug = augs[b]
            for cb in range(CB):
                sm = sms[(b, cb)]
                ms = smalls.tile([P, 1], F32, tag=f"ms{b}{cb}", bufs=1)
                nc.vector.tensor_mul(out=ms, in0=sm[:, 1:2], in1=sm[:, 0:1])
                # kvec partial: kv[r] = sum_c v[c,r] * (m*s)_c
                kv = ps_small.tile([32, 1], F32, tag="ps")
                nc.tensor.matmul(
                    kv, lhsT=vshs[b][:, 33 * cb:33 * cb + 32], rhs=ms,
                    start=True, stop=True)
                kvp = smalls.tile([R, 1], F32, tag=f"kv{b}{cb}", bufs=1)
                nc.vector.tensor_copy(out=kvp, in_=kv[0:R, :])
                kvsb[(b, cb)] = kvp
                # cst = shift - m*s
                cst = smalls.tile([P, 1], F32, tag=f"cst{b}{cb}", bufs=1)
                nc.vector.tensor_sub(
                    out=cst, in0=vshs[b][:, 33 * cb + 32:33 * cb + 33], in1=ms)
                csts[(b, cb)] = cst
                cst16 = smalls.tile([P, 1], F16, tag="cst16")
                nc.vector.tensor_copy(out=cst16, in_=cst)
                ctp = ps_small.tile([1, P], F16, tag="ps")
                nc.tensor.transpose(out=ctp, in_=cst16, identity=ident_h)
                cstrow = smalls.tile([1, P], F16, tag="cstrow")
                nc.scalar.copy(out=cstrow, in_=ctp)
                ctp4 = ps_small.tile([4, P], F32, tag="ps")
                nc.tensor.matmul(ctp4, lhsT=ones14, rhs=cstrow,
                                 start=True, stop=True)
                cstrow4 = smalls.tile([4, P], F16, tag="cstrow4")
                nc.scalar.copy(out=cstrow4, in_=ctp4)
                pstride = aug.ap[0][0]
                dst = bass.AP(
                    tensor=aug.tensor,
                    offset=aug.offset + R * pstride + P * cb,
                    ap=[[32 * pstride, 4], [1, P]])
                nc.gpsimd.dma_start(out=dst, in_=cstrow4)
            # kvecP: rows 32q+r = kvec[r], rows 32q+12 = -1 (evac's 0-kvecP
            # gives the +1 ones-row), rest 0
            krow = smalls.tile([16, 1], F32, tag=f"krow{b}", bufs=1)
            nc.gpsimd.dma_start(out=krow, in_=kinit[:, :])
            nc.vector.tensor_add(
                out=krow[0:R, :], in0=kvsb[(b, 0)], in1=kvsb[(b, 1)])
            kvecP = smalls.tile([P, 1], F32, tag=f"kvecP{b}", bufs=1)
            for q in range(4):
                nc.gpsimd.dma_start(out=kvecP[32 * q:32 * q + 16, :], in_=krow)
            kvecPs[b] = kvecP

        def emit_stage_a(b, ch):
            # vtx strips for chunks j = 4*ch + q; strip q covers partitions
            # 32q..32q+31 (rows 12..31 zero via the padded lhsT)
            if ch == 0:
                vtx_ps[b] = ps_vtx.tile([P, 1024], F32, tag="vtx",
                                        name=f"vtx{b}")
            vps = vtx_ps[b]
            for cb in range(CB):
                for q in range(4):
                    nc.tensor.matmul(
                        vps[32 * q:32 * q + 32, 512 * ch:512 * (ch + 1)],
                        lhsT=vss[(b, cb)],
                        rhs=x_ap(b, cb, 2048 * ch + 512 * q, 512),
                        start=(cb == 0), stop=(cb == CB - 1),
                        tile_position=(0, 32 * q),
                        skip_group_check=True)

        def emit_evac(b, ch):
            # vt = vtx - kvec; zero rows 32q+12 become +1 (kvecP=-1 there)
            vt = vtp.tile([P, 512], F16, tag="vt")
            nc.vector.tensor_scalar_sub(
                out=vt, in0=vtx_ps[b][:, 512 * ch:512 * (ch + 1)],
                scalar1=kvecPs[b])
            vts[(b, ch)] = vt

        def emit_unit(b, k, cb):
            # output unit [128,1024]: chunks (2k, 2k+1); vtx strips
            # q = 2k%4, (2k+1)%4 of vts[(b, k//2)]
            path = paths[b][2 * k + cb]
            aug = augs[b]
            sm = sms[(b, cb)]
            xap = x_ap(b, cb, 1024 * k, 1024)
            pm = ps_pm.tile([P, 1024], F32, tag="pm")
            vt = vts[(b, k // 2)]
            for j2 in range(2):
                q = (2 * k + j2) % 4
                pslice = pm[:, 512 * j2:512 * (j2 + 1)]
                if path == "A":
                    nc.tensor.matmul(
                        pslice, lhsT=diags[(b, cb)],
                        rhs=x_ap(b, cb, 1024 * k + 512 * j2, 512),
                        start=True, stop=False,
                        skip_group_check=True)
                    nc.tensor.matmul(
                        pslice,
                        lhsT=aug[32 * q:32 * q + R, P * cb:P * (cb + 1)],
                        rhs=vt[32 * q:32 * q + R, :],
                        start=False, stop=True,
                        tile_position=(32 * q, 0),
                        skip_group_check=True)
                else:
                    nc.tensor.matmul(
                        pslice,
                        lhsT=aug[32 * q:32 * q + R + 1, P * cb:P * (cb + 1)],
                        rhs=vt[32 * q:32 * q + R + 1, :],
                        start=True, stop=True,
                        tile_position=(32 * q, 0),
                        skip_group_check=True)
            osb = outp.tile([P, 1024], F16, tag="osb")
            if path == "A":
                nc.scalar.activation(
                    out=osb, in_=pm, func=AF.Identity,
                    bias=csts[(b, cb)], scale=1.0)
            elif path == "D":
                nc.vector.scalar_tensor_tensor(
                    out=osb, in0=xap, scalar=sm[:, 0:1], in1=pm,
                    op0=_MULT, op1=_ADD)
            else:  # E
                t = outp.tile([P, 1024], F16, tag="tsx", bufs=3)
                nc.vector.tensor_scalar(
                    out=t, in0=xap, scalar1=sm[:, 0:1], scalar2=0.0,
                    op0=_MULT, op1=_ADD)
                pmsb = outp.tile([P, 1024], F16, tag="pmsb", bufs=3)
                nc.scalar.activation(out=pmsb, in_=pm, func=AF.Identity)
                nc.gpsimd.tensor_add(out=osb, in0=t, in1=pmsb)
            nc.sync.dma_start(
                out=out_d[b, cb * P:(cb + 1) * P, 1024 * k:1024 * (k + 1)],
                in_=osb)

        # ================= schedule =================
        for cb in range(CB):
            emit_stats(0, cb, 0)
            emit_stats(0, cb, 1)
            emit_chain(0, cb)
        emit_cst(0)
        emit_stats(1, 0, 0)
        emit_stats(1, 0, 1)
        emit_stage_a(0, 0)
        emit_evac(0, 0)
        emit_gxred(1, 0)
        emit_stats(1, 1, 0)
        emit_stats(1, 1, 1)
        emit_gxred(1, 1)
        emit_stage_a(0, 1)
        emit_evac(0, 1)
        emit_chain(1, 0)
        emit_chain(1, 1)
        emit_cst(1)
        for k in range(4):
            for cb in range(CB):
                emit_unit(0, k, cb)
        emit_stage_a(1, 0)
        emit_evac(1, 0)
        emit_stage_a(1, 1)
        emit_evac(1, 1)
        for k in range(4):
            for cb in range(CB):
                emit_unit(1, k, cb)

    nc.finalize()
    return nc


def _host_prep(x, ccm_params):
    x = np.asarray(x, dtype=np.float32).reshape(B, C, HW).astype(np.float16)
    x = np.ascontiguousarray(x)
    cp = np.asarray(ccm_params, dtype=np.float32)
    u = cp[:, :C * R].reshape(B, C, R)
    v = cp[:, C * R:2 * C * R].reshape(B, C, R)
    shift = cp[:, 2 * C * R:].reshape(B, C)
    # aug: [B, 128, C] fp16; strips s=0..3: rows 32s..32s+11 = u^T,
    # row 32s+12 = cst written on device
    aug = np.zeros((B, P, C), np.float16)
    ut = u.transpose(0, 2, 1).astype(np.float16)
    for sx in range(4):
        aug[:, 32 * sx:32 * sx + R, :] = ut
    aug = np.ascontiguousarray(aug)
    # vsh: [B, CB, P, 33] f32: cols 0..11 = v, 12..31 zero pad, col 32 = shift
    vsh = np.zeros((B, CB, P, 33), np.float32)
    vsh[..., :R] = v.reshape(B, CB, P, R)
    vsh[..., 32] = shift.reshape(B, CB, P)
    vsh = np.ascontiguousarray(vsh)
    gmask = np.zeros((P, 16), np.float32)
    gmask[np.arange(P), np.arange(P) // GPC] = 1.0
    gmaskG = np.ascontiguousarray(gmask / GPC)
    gmaskT = np.ascontiguousarray(gmask.T)
    gmask16 = np.zeros((P, 32), np.float16)
    gmask16[:, :16] = gmask
    foldm = np.zeros((P, 16), np.float32)
    for q in range(4):
        foldm[32 * q + np.arange(16), np.arange(16)] = 1.0 / (GPC * HW)
    foldm = np.ascontiguousarray(foldm)
    ident16 = np.eye(P, dtype=np.float16)
    kinit = np.zeros((16, 1), np.float32)
    kinit[12, 0] = -1.0
    in_maps = []
    for c in range(N_CORES):
        bs = slice(c * BPC, (c + 1) * BPC)
        in_maps.append({
            "x": x[bs], "aug": aug[bs], "vsh": vsh[bs],
            "gmaskG": gmaskG, "gmask16": gmask16, "foldm": foldm,
            "gmaskT": gmaskT, "ident16": ident16,
            "kinit": kinit, "ones16": np.ones((1, 4), np.float16),
        })
    return in_maps


def kernel(x, ccm_params, _trace=False, _paths=DEF_PATHS, _s2=DEF_S2,
           **_ignored):
    in_maps = _host_prep(x, ccm_params)
    nc = build_nc(paths=_paths, s2cfg=_s2)
    res = run_bass_kernel_spmd(
        nc, in_maps, core_ids=list(range(N_CORES)), trace=_trace)
    out = np.concatenate([r["out"] for r in res.results], axis=0)
    out = out.reshape(B, C, H, W).astype(np.float32, copy=False)
    if _trace:
        return out, res
    return out


# revision 21
# speedup vs baseline: 1.1553x; 1.1553x over previous
"""CCMLite kernel for Trainium2: GroupNorm(affine=False) + low-rank channel mix.

out = x_norm + u @ (v^T @ x_norm) + shift, with x_norm = groupnorm(x).

Sharding: data-parallel over batch B=16 across 8 cores (2 batch elems/core).

DMA floor is ~24us/core (8.4 MB at ~360 GB/s). DVE/ACT are the only engines
that can read PSUM; latency ladders (stats -> group chain -> vs) gate each
batch, so they are kept short and high-priority:
  - all x loads issue up-front on the sync ring; the last tile of each batch
    is split in half so the gating stats start earlier
  - stats ops are sliced to 1024 columns so chain rungs never wait long
    behind bulk ops; s1 via DVE tensor_reduce (runs in DVE's idle load
    window) or ACT Copy+accum; s2 via ACT Square+accum or DVE STT+accum
  - group chain: 1/GPC is folded into the host-side mask so the gs matmul
    yields group means/E[x^2] directly; Rsqrt fuses sqrt+reciprocal
  - stage A (vtx = vs^T x) packs 4 rank-12 strips per PSUM tile via
    tile_position (they pipeline in distinct PE column groups); lhsT is
    zero-padded to 32 cols so strips cover all 128 partitions; the evac op
    subtracts kvec (mean correction) as a per-partition scalar and maps the
    zero rows 32q+12 to +1 (kvecP=-1 there), creating the ones-row that
    carries cst through stage-B matmuls
  - stage B units of [128,1024]: path D = DVE STT (s*x + pm) from PSUM,
    path A = PE diag(s) matmul + ACT Identity+bias, path E = DVE 4x t=s*x,
    ACT evac, Pool TT-add; per-unit stores stream on the sync ring
"""

from contextlib import ExitStack

import numpy as np

import concourse.bass as bass
import concourse.tile as tile
from concourse import bacc, mybir
from concourse.bass_utils import run_bass_kernel_spmd

N_CORES = 8
B, C, H, W = 16, 256, 64, 64
HW = H * W            # 4096
R = 12                # low rank
G = 32                # groups
GPC = C // G          # 8 channels per group
P = 128               # partitions
CB = C // P           # 2 channel blocks
BPC = B // N_CORES    # 2 batch elements per core
EPS = 1e-6
F32 = mybir.dt.float32
F16 = mybir.dt.float16

_MULT = mybir.AluOpType.mult
_ADD = mybir.AluOpType.add
AF = mybir.ActivationFunctionType

# ---- schedule knobs ----
# s2 per (b, cb): 'act' = ACT Square+accum, 'stt' = DVE STT+accum
# (s1 always rides tile_position-packed PE group-sum matmuls)
DEF_S2 = {(0, 0): "act", (0, 1): "stt", (1, 0): "act", (1, 1): "stt"}
# stage-B path per batch: 8 chars, unit order u = 2k+cb (k-major)
DEF_PATHS = ("DEADEDAE", "DAEDEDDE")


def build_nc(paths=DEF_PATHS, s2cfg=DEF_S2):
    nc = bacc.Bacc(None, target_bir_lowering=False)
    x_d = nc.dram_tensor("x", [BPC, C, HW], F16, kind="ExternalInput")
    aug_d = nc.dram_tensor("aug", [BPC, P, C], F16, kind="ExternalInput")
    vsh_d = nc.dram_tensor("vsh", [BPC, CB, P, 33], F32, kind="ExternalInput")
    gmaskG_d = nc.dram_tensor("gmaskG", [P, 16], F32, kind="ExternalInput")
    gmask16_d = nc.dram_tensor("gmask16", [P, 32], F16, kind="ExternalInput")
    foldm_d = nc.dram_tensor("foldm", [P, 16], F32, kind="ExternalInput")
    gmaskT_d = nc.dram_tensor("gmaskT", [16, P], F32, kind="ExternalInput")
    ident_d = nc.dram_tensor("ident16", [P, P], F16, kind="ExternalInput")
    repm_d = nc.dram_tensor("repm", [R, P], F32, kind="ExternalInput")
    negr_d = nc.dram_tensor("negr", [1, P], F32, kind="ExternalInput")
    ones_d = nc.dram_tensor("ones16", [1, 4], F16, kind="ExternalInput")
    out_d = nc.dram_tensor("out", [BPC, C, HW], F16, kind="ExternalOutput")

    with tile.TileContext(nc) as tc, ExitStack() as ctx:
        consts = ctx.enter_context(tc.tile_pool(name="consts", bufs=1))
        xbp = ctx.enter_context(tc.tile_pool(name="xbp", bufs=6))
        junkp = ctx.enter_context(tc.tile_pool(name="junkp", bufs=2))
        outp = ctx.enter_context(tc.tile_pool(name="outp", bufs=6))
        vtp = ctx.enter_context(tc.tile_pool(name="vtp", bufs=4))
        smalls = ctx.enter_context(tc.tile_pool(name="smalls", bufs=2))
        ps_small = ctx.enter_context(
            tc.tile_pool(name="ps_small", bufs=2, space="PSUM"))
        ps_vtx = ctx.enter_context(
            tc.tile_pool(name="ps_vtx", bufs=1, space="PSUM"))
        ps_pm = ctx.enter_context(tc.tile_pool(name="ps_pm", bufs=2, space="PSUM"))

        # ---- consts ----
        gmaskG = consts.tile([P, 16], F32)   # group mask * 1/GPC
        nc.gpsimd.dma_start(out=gmaskG, in_=gmaskG_d[:, :])
        gmask16 = consts.tile([P, 32], F16)  # group mask, zero-padded
        nc.gpsimd.dma_start(out=gmask16, in_=gmask16_d[:, :])
        foldm = consts.tile([P, 16], F32)    # strip fold mask / (GPC*HW)
        nc.gpsimd.dma_start(out=foldm, in_=foldm_d[:, :])
        gmaskT = consts.tile([16, P], F32)
        nc.gpsimd.dma_start(out=gmaskT, in_=gmaskT_d[:, :])
        ident_h = consts.tile([P, P], F16)
        nc.gpsimd.dma_start(out=ident_h, in_=ident_d[:, :])
        repmask = consts.tile([R, P], F32)
        nc.gpsimd.dma_start(out=repmask, in_=repm_d[:, :])
        negrow = consts.tile([1, P], F32)
        nc.gpsimd.dma_start(out=negrow, in_=negr_d[:, :])
        ones1f = consts.tile([1, 1], F32)
        nc.vector.memset(ones1f, 1.0)
        ones14 = consts.tile([1, 4], F16)
        nc.gpsimd.dma_start(out=ones14, in_=ones_d[:, :])
        eps_t = consts.tile([16, 1], F32)
        nc.vector.memset(eps_t, EPS)

        # warm ACT tables first so Sqrt/Square never table-load mid-kernel
        twarm = smalls.tile([16, 1], F32, tag="twarm", bufs=1)
        nc.scalar.activation(out=twarm, in_=eps_t, func=AF.Square)
        nc.scalar.activation(out=twarm, in_=eps_t, func=AF.Sqrt,
                             bias=eps_t[:, 0:1], scale=1.0)
        nc.scalar.activation(out=twarm, in_=eps_t, func=AF.Identity)
        nc.scalar.activation(out=twarm, in_=eps_t, func=AF.Copy)

        # per-batch params on the gpsimd SWDGE ring (Pool is idle early;
        # keeps the scalar sequencer free for the squares)
        augs, vshs = [], []
        for b in range(BPC):
            aug = smalls.tile([P, 2 * P], F16, tag=f"aug{b}", bufs=1)
            nc.gpsimd.dma_start(out=aug, in_=aug_d[b])
            vsh = smalls.tile([P, 66], F32, tag=f"vsh{b}", bufs=1)
            for cb in range(CB):
                nc.gpsimd.dma_start(
                    out=vsh[:, 33 * cb:33 * (cb + 1)], in_=vsh_d[b, cb])
            augs.append(aug)
            vshs.append(vsh)

        # ---- all x loads up-front, sync ring; last tile of each batch split
        # so the tail-gating stats start earlier ----
        # xparts[(b,cb,h)] = list of (tile, col0, width) covering cols 0..2048
        xparts = {}
        order = [(0, 0, 0), (0, 0, 1), (0, 1, 0), (0, 1, 1),
                 (1, 1, 0), (1, 1, 1), (1, 0, 0), (1, 0, 1)]
        for b, cb, h in order:
                    eng = nc.sync if b == 0 or cb == 1 else nc.gpsimd
                    if cb == 1 and h == 1:
                        parts = []
                        for i in range(2):
                            tb = xbp.tile([P, 1024], F16, tag="xbt2",
                                          bufs=4, name=f"xl{b}{i}")
                            eng.dma_start(
                                out=tb,
                                in_=x_d[b, cb * P:(cb + 1) * P,
                                        h * 2048 + 1024 * i:
                                        h * 2048 + 1024 * (i + 1)])
                            parts.append((tb, 1024 * i, 1024))
                        xparts[(b, cb, h)] = parts
                    else:
                        tb = xbp.tile([P, 2048], F16, tag="xbt",
                                      name=f"x{b}{cb}{h}")
                        eng.dma_start(
                            out=tb,
                            in_=x_d[b, cb * P:(cb + 1) * P,
                                    h * 2048:(h + 1) * 2048])
                        xparts[(b, cb, h)] = [(tb, 0, 2048)]

        def x_ap(b, cb, col0, width):
            h, c = col0 // 2048, col0 % 2048
            for tile_, t0, tw in xparts[(b, cb, h)]:
                if t0 <= c and c + width <= t0 + tw:
                    return tile_[:, c - t0:c - t0 + width]
            raise AssertionError((b, cb, col0, width))

        def x_slices(b, cb, h):
            # 1024-wide (tile, slice) pieces of column range [2048h, 2048h+2048)
            out = []
            for tile_, t0, tw in xparts[(b, cb, h)]:
                for i in range(tw // 1024):
                    out.append(tile_[:, 1024 * i:1024 * (i + 1)])
            return out

        sms = {}     # (b,cb) -> [128,2] f32: col0 rstd, col1 group-mean
        vss = {}     # (b,cb) -> [128,32] f16 (v*s, zero-padded)
        diags = {}   # (b,cb) -> [128,128] f16 diag(s)
        kvsb = {}    # (b,cb) -> [R,1] f32 partial kvec
        kvecPs = {}  # b -> [128,1] f32 strip-replicated kvec (rows 32q+12=-1)
        accs = {}    # (b,cb) -> [128,4] f32 s2 accumulator columns
        naccs = {}   # (b,cb) -> number of s2 cols used
        gxs = {}     # (b,cb) -> [128,512] PSUM packed group partial sums
        csts = {}
        vts = {}
        vtx_ps = {}

        def get_acc(b, cb):
            if (b, cb) not in accs:
                accs[(b, cb)] = smalls.tile(
                    [P, 4], F32, tag=f"acc{b}{cb}", bufs=1, name=f"acc{b}{cb}")
                naccs[(b, cb)] = 0
            return accs[(b, cb)]

        def emit_stats(b, cb, h):
            # s1: packed group-sum matmuls -- strip q of the shared [128,512]
            # PSUM tile accumulates chunks q and q+4 at PE column 32q
            if h == 0:
                gxs[(b, cb)] = ps_small.tile([P, 512], F32, tag="ps",
                                             name=f"gx{b}{cb}")
            gx = gxs[(b, cb)]
            for q in range(4):
                nc.tensor.matmul(
                    gx[32 * q:32 * q + 32, :], lhsT=gmask16,
                    rhs=x_ap(b, cb, 2048 * h + 512 * q, 512),
                    start=(h == 0), stop=(h == 1),
                    tile_position=(0, 32 * q),
                    skip_group_check=True)
            # s2 per natural tile piece
            acc = get_acc(b, cb)
            if s2cfg[(b, cb)] == "act":
                for tile_, t0, tw in xparts[(b, cb, h)]:
                    col = naccs[(b, cb)]
                    naccs[(b, cb)] += 1
                    ja = junkp.tile([P, 2048], F16, tag="ja")
                    nc.scalar.activation(
                        out=ja[:, 0:tw], in_=tile_, func=AF.Square,
                        accum_out=acc[:, col:col + 1])
            else:
                # sliced to 1024 so chain-ladder rungs never wait long
                for sl in x_slices(b, cb, h):
                    col = naccs[(b, cb)]
                    naccs[(b, cb)] += 1
                    jd = junkp.tile([P, 1024], F16, tag="jd")
                    nc.vector.scalar_tensor_tensor(
                        out=jd, in0=sl, scalar=1.0, in1=sl,
                        op0=_MULT, op1=_MULT, accum_out=acc[:, col:col + 1])

        def fold(acc, n, out):
            # out = sum(acc[:, 0:n]) / HW
            if n == 2:
                nc.vector.tensor_scalar(
                    out=out, in0=acc[:, 0:1], scalar1=acc[:, 1:2],
                    scalar2=1.0 / HW, op0=_ADD, op1=_MULT)
            elif n == 3:
                nc.vector.tensor_scalar(
                    out=acc[:, 0:1], in0=acc[:, 0:1], scalar1=acc[:, 1:2],
                    scalar2=acc[:, 2:3], op0=_ADD, op1=_ADD)
                nc.vector.tensor_scalar_mul(
                    out=out, in0=acc[:, 0:1], scalar1=1.0 / HW)
            else:
                assert n == 4
                nc.vector.tensor_scalar(
                    out=acc[:, 0:1], in0=acc[:, 0:1], scalar1=acc[:, 1:2],
                    scalar2=acc[:, 2:3], op0=_ADD, op1=_ADD)
                nc.vector.tensor_scalar(
                    out=out, in0=acc[:, 0:1], scalar1=acc[:, 3:4],
                    scalar2=1.0 / HW, op0=_ADD, op1=_MULT)

        gxrs = {}

        def emit_gxred(b, cb):
            # consume the gx PSUM tile early so the ps ring can rotate
            gxr = smalls.tile([P, 1], F32, tag="gxr", bufs=4,
                              name=f"gxr{b}{cb}")
            nc.vector.tensor_reduce(
                out=gxr, in_=gxs[(b, cb)], axis=mybir.AxisListType.X, op=_ADD)
            gxrs[(b, cb)] = gxr

        def emit_chain(b, cb):
            # ladder: mg(mm) / e2 fold -> gs(mm) -> var -> sqrt ->
            # recip -> bc(mm) -> sm -> vs
            if (b, cb) not in gxrs:
                emit_gxred(b, cb)
            mg = ps_small.tile([16, 1], F32, tag="ps")
            nc.tensor.matmul(mg, lhsT=foldm, rhs=gxrs[(b, cb)],
                             start=True, stop=True)
            acc = accs[(b, cb)]
            msum = smalls.tile([P, 1], F32, tag="msum")
            fold(acc, naccs[(b, cb)], msum)
            gs = ps_small.tile([16, 1], F32, tag="ps")
            nc.tensor.matmul(gs, lhsT=gmaskG, rhs=msum, start=True, stop=True)
            gvals = smalls.tile([16, 2], F32, tag="gvals")
            tmpg = smalls.tile([16, 2], F32, tag="tmpg")
            nc.vector.tensor_copy(out=gvals[:, 1:2], in_=mg)
            nc.vector.tensor_mul(out=tmpg[:, 0:1], in0=gvals[:, 1:2],
                                 in1=gvals[:, 1:2])
            nc.vector.tensor_sub(out=tmpg[:, 1:2], in0=gs,
                                 in1=tmpg[:, 0:1])
            gsd = smalls.tile([16, 1], F32, tag="gsd")
            nc.scalar.activation(
                out=gsd, in_=tmpg[:, 1:2], func=AF.Sqrt,
                bias=eps_t[:, 0:1], scale=1.0)
            nc.vector.reciprocal(out=gvals[:, 0:1], in_=gsd)
            bc = ps_small.tile([P, 2], F32, tag="ps")
            nc.tensor.matmul(bc, lhsT=gmaskT, rhs=gvals, start=True, stop=True)
            sm = smalls.tile([P, 2], F32, tag=f"sm{b}{cb}", bufs=1)
            nc.vector.tensor_copy(out=sm, in_=bc)
            sms[(b, cb)] = sm
            # vs = v * s (fp16, zero-padded cols 12..31 so stage A strips
            # cover all 128 partitions)
            vs = smalls.tile([P, 32], F16, tag=f"vs{b}{cb}", bufs=1)
            nc.vector.tensor_scalar_mul(
                out=vs, in0=vshs[b][:, 33 * cb:33 * cb + 32],
                scalar1=sm[:, 0:1])
            vss[(b, cb)] = vs
            if "A" in paths[b]:
                diag = smalls.tile([P, P], F16, tag=f"diag{b}{cb}", bufs=1)
                nc.vector.tensor_scalar_mul(out=diag, in0=ident_h,
                                            scalar1=sm[:, 0:1])
                diags[(b, cb)] = diag

        def emit_cst(b):
            aug = augs[b]
            for cb in range(CB):
                sm = sms[(b, cb)]
                ms = smalls.tile([P, 1], F32, tag=f"ms{b}{cb}", bufs=1)
                nc.vector.tensor_mul(out=ms, in0=sm[:, 1:2], in1=sm[:, 0:1])
                # kvec partial: kv[r] = sum_c v[c,r] * (m*s)_c
                kv = ps_small.tile([32, 1], F32, tag="ps")
                nc.tensor.matmul(
                    kv, lhsT=vshs[b][:, 33 * cb:33 * cb + 32], rhs=ms,
                    start=True, stop=True)
                kvp = smalls.tile([R, 1], F32, tag=f"kv{b}{cb}", bufs=1)
                nc.vector.tensor_copy(out=kvp, in_=kv[0:R, :])
                kvsb[(b, cb)] = kvp
                # cst = shift - m*s
                cst = smalls.tile([P, 1], F32, tag=f"cst{b}{cb}", bufs=1)
                nc.vector.tensor_sub(
                    out=cst, in0=vshs[b][:, 33 * cb + 32:33 * cb + 33], in1=ms)
                csts[(b, cb)] = cst
                cst16 = smalls.tile([P, 1], F16, tag="cst16")
                nc.vector.tensor_copy(out=cst16, in_=cst)
                ctp = ps_small.tile([1, P], F16, tag="ps")
                nc.tensor.transpose(out=ctp, in_=cst16, identity=ident_h)
                cstrow = smalls.tile([1, P], F16, tag="cstrow")
                nc.scalar.copy(out=cstrow, in_=ctp)
                ctp4 = ps_small.tile([4, P], F32, tag="ps")
                nc.tensor.matmul(ctp4, lhsT=ones14, rhs=cstrow,
                                 start=True, stop=True)
                cstrow4 = smalls.tile([4, P], F16, tag="cstrow4")
                nc.scalar.copy(out=cstrow4, in_=ctp4)
                pstride = aug.ap[0][0]
                dst = bass.AP(
                    tensor=aug.tensor,
                    offset=aug.offset + R * pstride + P * cb,
                    ap=[[32 * pstride, 4], [1, P]])
                nc.gpsimd.dma_start(out=dst, in_=cstrow4)
            # kvecP: rows 32q+r = kvec[r], rows 32q+12 = -1 (evac's 0-kvecP
            # gives the +1 ones-row), rest 0 -- built by two tiny PE matmuls
            kvs = smalls.tile([R, 1], F32, tag=f"kvs{b}", bufs=1)
            nc.vector.tensor_add(out=kvs, in0=kvsb[(b, 0)], in1=kvsb[(b, 1)])
            kvP_ps = ps_small.tile([P, 1], F32, tag="ps")
            nc.tensor.matmul(kvP_ps, lhsT=repmask, rhs=kvs,
                             start=True, stop=False, skip_group_check=True)
            nc.tensor.matmul(kvP_ps, lhsT=negrow, rhs=ones1f,
                             start=False, stop=True, skip_group_check=True)
            kvecP = smalls.tile([P, 1], F32, tag=f"kvecP{b}", bufs=1)
            nc.vector.tensor_copy(out=kvecP, in_=kvP_ps)
            kvecPs[b] = kvecP

        def emit_stage_a(b, ch):
            # vtx strips for chunks j = 4*ch + q; strip q covers partitions
            # 32q..32q+31 (rows 12..31 zero via the padded lhsT)
            if ch == 0:
                vtx_ps[b] = ps_vtx.tile([P, 1024], F32, tag="vtx",
                                        name=f"vtx{b}")
            vps = vtx_ps[b]
            for cb in range(CB):
                for q in range(4):
                    nc.tensor.matmul(
                        vps[32 * q:32 * q + 32, 512 * ch:512 * (ch + 1)],
                        lhsT=vss[(b, cb)],
                        rhs=x_ap(b, cb, 2048 * ch + 512 * q, 512),
                        start=(cb == 0), stop=(cb == CB - 1),
                        tile_position=(0, 32 * q),
                        skip_group_check=True)

        def emit_evac(b, ch):
            # vt = vtx - kvec; zero rows 32q+12 become +1 (kvecP=-1 there)
            vt = vtp.tile([P, 512], F16, tag="vt")
            nc.vector.tensor_scalar_sub(
                out=vt, in0=vtx_ps[b][:, 512 * ch:512 * (ch + 1)],
                scalar1=kvecPs[b])
            vts[(b, ch)] = vt

        def emit_unit(b, k, cb):
            # output unit [128,1024]: chunks (2k, 2k+1); vtx strips
            # q = 2k%4, (2k+1)%4 of vts[(b, k//2)]
            path = paths[b][2 * k + cb]
            aug = augs[b]
            sm = sms[(b, cb)]
            xap = x_ap(b, cb, 1024 * k, 1024)
            pm = ps_pm.tile([P, 1024], F32, tag="pm")
            vt = vts[(b, k // 2)]
            for j2 in range(2):
                q = (2 * k + j2) % 4
                pslice = pm[:, 512 * j2:512 * (j2 + 1)]
                if path == "A":
                    nc.tensor.matmul(
                        pslice, lhsT=diags[(b, cb)],
                        rhs=x_ap(b, cb, 1024 * k + 512 * j2, 512),
                        start=True, stop=False,
                        skip_group_check=True)
                    nc.tensor.matmul(
                        pslice,
                        lhsT=aug[32 * q:32 * q + R, P * cb:P * (cb + 1)],
                        rhs=vt[32 * q:32 * q + R, :],
                        start=False, stop=True,
                        tile_position=(32 * q, 0),
                        skip_group_check=True)
                else:
                    nc.tensor.matmul(
                        pslice,
                        lhsT=aug[32 * q:32 * q + R + 1, P * cb:P * (cb + 1)],
                        rhs=vt[32 * q:32 * q + R + 1, :],
                        start=True, stop=True,
                        tile_position=(32 * q, 0),
                        skip_group_check=True)
            osb = outp.tile([P, 1024], F16, tag="osb")
            if path == "A":
                nc.scalar.activation(
                    out=osb, in_=pm, func=AF.Identity,
                    bias=csts[(b, cb)], scale=1.0)
            elif path == "D":
                nc.vector.scalar_tensor_tensor(
                    out=osb, in0=xap, scalar=sm[:, 0:1], in1=pm,
                    op0=_MULT, op1=_ADD)
            else:  # E
                t = outp.tile([P, 1024], F16, tag="tsx", bufs=3)
                nc.vector.tensor_scalar(
                    out=t, in0=xap, scalar1=sm[:, 0:1], scalar2=0.0,
                    op0=_MULT, op1=_ADD)
                pmsb = outp.tile([P, 1024], F16, tag="pmsb", bufs=3)
                nc.scalar.activation(out=pmsb, in_=pm, func=AF.Identity)
                nc.gpsimd.tensor_add(out=osb, in0=t, in1=pmsb)
            nc.sync.dma_start(
                out=out_d[b, cb * P:(cb + 1) * P, 1024 * k:1024 * (k + 1)],
                in_=osb)

        # ================= schedule =================
        for cb in range(CB):
            emit_stats(0, cb, 0)
            emit_stats(0, cb, 1)
            emit_chain(0, cb)
        emit_cst(0)
        emit_stats(1, 0, 0)
        emit_stats(1, 0, 1)
        emit_stage_a(0, 0)
        emit_evac(0, 0)
        emit_gxred(1, 0)
        emit_stats(1, 1, 0)
        emit_stats(1, 1, 1)
        emit_gxred(1, 1)
        emit_stage_a(0, 1)
        emit_evac(0, 1)
        emit_chain(1, 0)
        emit_chain(1, 1)
        emit_cst(1)
        for k in range(4):
            for cb in range(CB):
                emit_unit(0, k, cb)
        emit_stage_a(1, 0)
        emit_evac(1, 0)
        emit_stage_a(1, 1)
        emit_evac(1, 1)
        for k in range(4):
            for cb in range(CB):
                emit_unit(1, k, cb)

    nc.finalize()
    return nc


def _host_prep(x, ccm_params):
    x = np.asarray(x, dtype=np.float32).reshape(B, C, HW).astype(np.float16)
    x = np.ascontiguousarray(x)
    cp = np.asarray(ccm_params, dtype=np.float32)
    u = cp[:, :C * R].reshape(B, C, R)
    v = cp[:, C * R:2 * C * R].reshape(B, C, R)
    shift = cp[:, 2 * C * R:].reshape(B, C)
    # aug: [B, 128, C] fp16; strips s=0..3: rows 32s..32s+11 = u^T,
    # row 32s+12 = cst written on device
    aug = np.zeros((B, P, C), np.float16)
    ut = u.transpose(0, 2, 1).astype(np.float16)
    for sx in range(4):
        aug[:, 32 * sx:32 * sx + R, :] = ut
    aug = np.ascontiguousarray(aug)
    # vsh: [B, CB, P, 33] f32: cols 0..11 = v, 12..31 zero pad, col 32 = shift
    vsh = np.zeros((B, CB, P, 33), np.float32)
    vsh[..., :R] = v.reshape(B, CB, P, R)
    vsh[..., 32] = shift.reshape(B, CB, P)
    vsh = np.ascontiguousarray(vsh)
    gmask = np.zeros((P, 16), np.float32)
    gmask[np.arange(P), np.arange(P) // GPC] = 1.0
    gmaskG = np.ascontiguousarray(gmask / GPC)
    gmaskT = np.ascontiguousarray(gmask.T)
    gmask16 = np.zeros((P, 32), np.float16)
    gmask16[:, :16] = gmask
    foldm = np.zeros((P, 16), np.float32)
    for q in range(4):
        foldm[32 * q + np.arange(16), np.arange(16)] = 1.0 / (GPC * HW)
    foldm = np.ascontiguousarray(foldm)
    ident16 = np.eye(P, dtype=np.float16)
    repm = np.zeros((R, P), np.float32)
    negr = np.zeros((1, P), np.float32)
    for q in range(4):
        repm[np.arange(R), 32 * q + np.arange(R)] = 1.0
        negr[0, 32 * q + R] = -1.0
    in_maps = []
    for c in range(N_CORES):
        bs = slice(c * BPC, (c + 1) * BPC)
        in_maps.append({
            "x": x[bs], "aug": aug[bs], "vsh": vsh[bs],
            "gmaskG": gmaskG, "gmask16": gmask16, "foldm": foldm,
            "gmaskT": gmaskT, "ident16": ident16,
            "repm": repm, "negr": negr,
            "ones16": np.ones((1, 4), np.float16),
        })
    return in_maps


def kernel(x, ccm_params, _trace=False, _paths=DEF_PATHS, _s2=DEF_S2,
           **_ignored):
    in_maps = _host_prep(x, ccm_params)
    nc = build_nc(paths=_paths, s2cfg=_s2)
    res = run_bass_kernel_spmd(
        nc, in_maps, core_ids=list(range(N_CORES)), trace=_trace)
    out = np.concatenate([r["out"] for r in res.results], axis=0)
    out = out.reshape(B, C, H, W).astype(np.float32, copy=False)
    if _trace:
        return out, res
    return out
